# revision 45
# baseline (speedup 1.0000x reference)
"""Multi-head attention TRN2 kernel, 8-core (batch x head-block) sharded.

Problem (hardcoded): x[2,2048,1024] f32, Wq/Wk/Wv[1024,1024], Wo[1024,1024],
16 heads, dh=64. Reference computes softmax(Q K^T)/sqrt(1024) @ V @ Wo with the
division AFTER softmax (folded here into Wo as a host-side 1/32 scale).

Sharding: core c handles batch b=c//4 and head block hb=c%4 (4 heads = 256 dims:
Wq/Wk/Wv column slice, Wo row slice). Each core emits a partial Y[2048,1024]
in bf16; host sums the 4 partials per batch in f32.

Kernel structure (all operands bf16, PSUM accumulation f32):
- QKV projections produce Q^T,K^T dim-major ([128 dims of a 2-head group, S])
  and V token-major into 128-col blocks whose cols 64:128 are ones, so the
  PV matmul also produces the softmax denominator (rows 64:128 of pO).
- Scores use PE array packing: the two heads of a group contract their 64
  dims on row-halves of the PE via tile_position=(0,0)/(64,0), concurrently.
- exp on ACT reads [128,1024] PSUM tiles (scores for both heads of a group,
  512 queries); ACT is the critical path (~147us) and everything else is
  emission-ordered to hide under it.
- 1/den via reciprocal_approx_fast (single custom DVE op, ~51 ULP).
"""

import numpy as np
import ml_dtypes

import concourse.tile as tile
from concourse import bacc, mybir
from concourse.bass_utils import run_bass_kernel_spmd

N_CORES = 8
B = 2
S = 2048          # tokens per batch (= per core)
D = 1024          # model dim
DH = 64           # head dim
HPC = 4           # heads per core
DL = HPC * DH     # 256 local output dims per core
NG = DL // 128    # 2 partition groups of local dims (head pairs)
NK = D // 128     # 8 k-strips for QKV contraction
NT = S // 128     # 16 key strips
NSH = 4           # query blocks of 512 per group
QB = S // NSH     # 512 queries per block
VW = 128          # V block: cols 0:64 = V dims, cols 64:128 = ones (denoms)

F32 = mybir.dt.float32
# fp16 (not bf16): same 1 cycle/col PE rate, 2x DVE rate, 10-bit mantissa.
# exp is computed as exp(s + EXPB) so its fp16 output can't overflow
# (scores reach ~18.5; fp16 max 65504); the shift cancels in softmax.
BF = mybir.dt.float16
EXPB = -12.0
EXP = mybir.ActivationFunctionType.Exp
MULT = mybir.AluOpType.mult

DEBUG = False
ROWTILE = True   # PE array packing for the K=64 score matmuls
VHEAD = True     # emit V GEMMs interleaved after the first scores+exp
WARM = True      # prologue exp to trigger the ACT table load early
SEQ = False      # fully sequential emission (no phase overlap; debug)


def build_nc():
    nc = bacc.Bacc("TRN2", target_bir_lowering=False, debug=False)
    xT = nc.declare_dram_parameter("xT", [D, S], BF, isOutput=False)
    Wq = nc.declare_dram_parameter("Wq", [D, DL], BF, isOutput=False)
    Wk = nc.declare_dram_parameter("Wk", [D, DL], BF, isOutput=False)
    Wv = nc.declare_dram_parameter("Wv", [D, DL], BF, isOutput=False)
    Wo = nc.declare_dram_parameter("Wo", [DL, D], BF, isOutput=False)
    Yp = nc.declare_dram_parameter("Yp", [S, D], BF, isOutput=True)
    dumps = {}
    if DEBUG:
        for nm, shp in (("dq", [128, NG * S]), ("dk", [128, NG * S]),
                        ("dv", [128, HPC * NT * VW]), ("do", [128, NG * S]),
                        ("dd", [128, NG * S]), ("du", [128, NG * S]),
                        ("de", [128, 16 * 1024])):
            dumps[nm] = nc.declare_dram_parameter(nm, shp, BF, isOutput=True)

    with tile.TileContext(nc) as tc:
        with tc.tile_pool(name="singles", bufs=1) as singles:
            wq_sb = singles.tile([128, NK * NG * 128], BF)
            wk_sb = singles.tile([128, NK * NG * 128], BF)
            wv_sb = singles.tile([128, NK * DL], BF)
            wo_sb = singles.tile([128, NG * D], BF)
            qt_sb = singles.tile([128, NG * S], BF)
            # rowtile: [128 dims of group g, g*S+keys]. fallback: per-head
            # zero-padded blocks [128, h*S+keys] (dims in rows (h%2)*64..+64)
            kt_sb = singles.tile([128, (NG if ROWTILE else HPC) * S], BF)
            ot_sb = singles.tile([128, NG * S], BF)
            vaug_sb = singles.tile([128, HPC * NT * VW], BF)
            xall = singles.tile([128, 4 * NK * 512], BF)  # blocks (c*NK+k)
            warm = singles.tile([128, 1], F32)
            biast = singles.tile([128, 1], F32)
            spin = singles.tile([128, 256], BF)  # never written: junk is fine
            if DEBUG:
                dden_sb = singles.tile([128, NG * S], BF)
                dunn_sb = singles.tile([128, NG * S], BF)
                dexp_sb = singles.tile([128, 16 * 1024], BF)

            # ---- DMA prologue: batched multi-dim-AP transfers (one per
            # tensor/chunk) — per-DMA queue overhead (~600ns) would
            # otherwise serialize ~80 small DMAs for ~50us.
            def dma_chunk(c):
                nc.sync.dma_start(
                    out=xall[:, c * NK * 512:(c + 1) * NK * 512].rearrange(
                        "p (k c) -> p k c", k=NK),
                    in_=xT[:, c * 512:(c + 1) * 512].rearrange(
                        "(k p) c -> p k c", p=128),
                )

            dma_chunk(0)
            for w_dram, w_sb in ((Wk, wk_sb), (Wq, wq_sb)):
                nc.sync.dma_start(
                    out=w_sb[:].rearrange("p (k g c) -> p k g c", k=NK, g=NG),
                    in_=w_dram[:, :].rearrange("(k p) (g c) -> p k g c",
                                               p=128, g=NG),
                )
            nc.sync.dma_start(
                out=wv_sb[:].rearrange("p (k c) -> p k c", k=NK),
                in_=Wv[:, :].rearrange("(k p) c -> p k c", p=128),
            )
            dma_chunk(1)
            dma_chunk(2)
            dma_chunk(3)
            nc.sync.dma_start(
                out=wo_sb[:].rearrange("p (g c) -> p g c", g=NG),
                in_=Wo[:, :].rearrange("(g p) c -> p g c", p=128),
            )

            nc.vector.memset(spin[:], 0.5)
            nc.vector.memset(biast[:], EXPB)
            # warm the ACT exp table during the prologue (table load ~2.7us)
            if WARM:
                nc.vector.memset(warm[:], 0.0)
                nc.scalar.activation(warm[:], warm[:], EXP)
            # ones in all V blocks; the V evacuations overwrite cols 0:64
            nc.vector.memset(vaug_sb[:], 1.0)
            if not ROWTILE:
                nc.vector.memset(kt_sb[:], 0.0)

            def proj_chunk(w_sb, g, c):
                ps = auxp.tile([128, 512], F32, name="ps_aux")
                for k in range(NK):
                    cb = (k * NG + g) * 128
                    nc.tensor.matmul(
                        ps[:],
                        w_sb[:, cb:cb + 128],
                        xall[:, (c * NK + k) * 512:(c * NK + k + 1) * 512],
                        start=(k == 0),
                        stop=(k == NK - 1),
                    )
                return ps

            def q_chunk(g, c):
                ps = proj_chunk(wq_sb, g, c)
                nc.vector.tensor_copy(
                    out=qt_sb[:, g * S + c * 512:g * S + (c + 1) * 512],
                    in_=ps[:],
                )

            def k_chunk(g, c):
                # K^T for group g from token chunk c (keys c*512..+512)
                ps = proj_chunk(wk_sb, g, c)
                if ROWTILE:
                    nc.vector.tensor_copy(
                        out=kt_sb[:, g * S + c * 512:g * S + (c + 1) * 512],
                        in_=ps[:],
                    )
                else:
                    for hh in range(2):
                        rows = slice(hh * 64, (hh + 1) * 64)
                        nc.vector.tensor_copy(
                            out=kt_sb[rows, (2 * g + hh) * S + c * 512:
                                      (2 * g + hh) * S + (c + 1) * 512],
                            in_=ps[rows, :],
                        )

            def v_chunk(c):
                # V token-major into vaug blocks (all 4 heads)
                for t in range(4):
                    j = c * 4 + t
                    pv = auxp.tile([128, 512], F32, name="ps_aux")
                    xb = (c * NK) * 512 + t * 128
                    for k in range(NK):
                        nc.tensor.matmul(
                            pv[:, 0:DL],
                            xall[:, xb + k * 512:xb + k * 512 + 128],
                            wv_sb[:, k * DL:(k + 1) * DL],
                            start=(k == 0),
                            stop=(k == NK - 1),
                        )
                    for h in range(HPC):
                        nc.vector.tensor_copy(
                            out=vaug_sb[:, (h * NT + j) * VW:
                                        (h * NT + j) * VW + DH],
                            in_=pv[:, h * DH:(h + 1) * DH],
                        )

            def scores_pair(g, sh, j, pS_t):
                # both heads of group g concurrently on PE row-halves
                q0 = g * S + sh * QB
                for hh in range(2):
                    if ROWTILE:
                        rows = slice(hh * 64, (hh + 1) * 64)
                        nc.tensor.matmul(
                            pS_t[:, hh * 512:(hh + 1) * 512],
                            kt_sb[rows, g * S + j * 128:g * S + (j + 1) * 128],
                            qt_sb[rows, q0:q0 + QB],
                            tile_position=(hh * 64, 0),
                        )
                    else:
                        h = 2 * g + hh
                        nc.tensor.matmul(
                            pS_t[:, hh * 512:(hh + 1) * 512],
                            kt_sb[:, h * S + j * 128:h * S + (j + 1) * 128],
                            qt_sb[:, q0:q0 + QB],
                        )

            def pv_pair(g, j, pO, exps):
                for hh in range(2):
                    vb = ((g * 2 + hh) * NT + j) * VW
                    nc.tensor.matmul(
                        pO[hh][:],
                        vaug_sb[:, vb:vb + VW],
                        exps[:, hh * 512:(hh + 1) * 512],
                        start=(j == 0),
                        stop=(j == NT - 1),
                        skip_group_check=True,
                    )

            def norm(g, sh, pO):
                # pO rows 64:128 hold the softmax denominator (ones cols of
                # vaug), replicated across 64 partitions.
                # NOTE: reciprocal_approx_fast (custom DVE ucode) silently
                # corrupts on partial-partition APs — gather both heads'
                # denominators into one full-128-partition tile first.
                q0 = g * S + sh * QB
                dn = normp.tile([128, 512], F32, name="dn")
                rb = normp.tile([128, 512], F32, name="rb")
                for hh in range(2):
                    nc.vector.tensor_copy(
                        out=dn[hh * 64:(hh + 1) * 64, :],
                        in_=pO[hh][64:128, :])
                    if DEBUG:
                        nc.vector.tensor_copy(
                            out=dden_sb[hh * 64:(hh + 1) * 64, q0:q0 + QB],
                            in_=pO[hh][64:128, :])
                        nc.vector.tensor_copy(
                            out=dunn_sb[hh * 64:(hh + 1) * 64, q0:q0 + QB],
                            in_=pO[hh][0:DH, :])
                nc.vector.reciprocal_approx_fast(out=rb[:], in_=dn[:])
                for hh in range(2):
                    nc.vector.tensor_tensor(
                        out=ot_sb[hh * 64:(hh + 1) * 64, q0:q0 + QB],
                        in0=pO[hh][0:DH, :],
                        in1=rb[hh * 64:(hh + 1) * 64, :],
                        op=MULT,
                    )

            def attn(g, sh, feeders=None):
                # feeders: {slot: emission_fn} interleaved into the strip
                # loop. Slot -1 fires after the first scores+exp; slot jj
                # fires at the top of iteration jj. Feeder emissions that
                # WRITE data read by later strips (kt chunks, vaug blocks)
                # MUST be emitted before those strips — Tile only tracks
                # deps against writes already emitted.
                feeders = feeders or {}
                dump_exp = DEBUG and g == 0 and sh == 0
                pO = [pOp.tile([128, 512], F32, name="pO") for _ in range(2)]
                exps = {}

                def emit_strip(j):
                    pS_t = pSp.tile([128, 1024], F32, name="pS")
                    scores_pair(g, sh, j, pS_t)
                    exps[j] = expp.tile([128, 1024], BF, name="expst")
                    nc.scalar.activation(exps[j][:], pS_t[:], EXP, bias=biast[:])
                    if dump_exp:
                        nc.vector.tensor_copy(
                            out=dexp_sb[:, j * 1024:(j + 1) * 1024],
                            in_=exps[j][:])

                # Scores/exp for the next strips are emitted ahead of this
                # iteration's PV + feeder work so the ACT exp stream (the
                # critical path) is never queued behind PE work it doesn't
                # depend on.
                for j in (0, 1):
                    emit_strip(j)
                for jj in range(0, NT, 2):
                    for j in (jj + 2, jj + 3):
                        if j < NT:
                            emit_strip(j)
                    for fn in feeders.get(jj, ()):
                        fn()
                    for j in (jj, jj + 1):
                        pv_pair(g, j, pO, exps.pop(j))
                norm(g, sh, pO)

            def proj(ts):
                for t in ts:
                    ysb = ysbp.tile([128, 1024], BF, name="ysb")
                    for e in range(2):
                        pY = auxp.tile([128, 512], F32, name="ps_aux")
                        for g in range(NG):
                            nc.tensor.matmul(
                                pY[:],
                                ot_sb[:, g * S + t * 128:g * S + (t + 1) * 128],
                                wo_sb[:, g * D + e * 512:g * D + (e + 1) * 512],
                                start=(g == 0),
                                stop=(g == NG - 1),
                            )
                        nc.vector.tensor_copy(
                            out=ysb[:, e * 512:(e + 1) * 512], in_=pY[:])
                    nc.sync.dma_start(
                        out=Yp[t * 128:(t + 1) * 128, :], in_=ysb[:])

            with tc.tile_pool(name="pS", bufs=2, space="PSUM") as pSp, \
                 tc.tile_pool(name="pO", bufs=3, space="PSUM") as pOp, \
                 tc.tile_pool(name="aux", bufs=1, space="PSUM") as auxp, \
                 tc.tile_pool(name="expp", bufs=4) as expp, \
                 tc.tile_pool(name="normp", bufs=2) as normp, \
                 tc.tile_pool(name="ysbp", bufs=4) as ysbp:
                if SEQ:
                    for g in range(2):
                        for c in range(4):
                            k_chunk(g, c)
                            q_chunk(g, c)
                    for c in range(4):
                        v_chunk(c)
                    for g in range(2):
                        for sh in range(4):
                            attn(g, sh)
                    proj(range(0, 16))
                else:
                    # PE warm-up during the DMA prologue: HAM un-throttles
                    # (1.2 -> 2.4 GHz) only after ~3.4us of sustained PE
                    # activity; these dummy matmuls on junk SBUF ensure the
                    # real work starts at full clock.
                    for _ in range(16):
                        sp = auxp.tile([128, 512], F32, name="ps_aux")
                        nc.tensor.matmul(sp[:, 0:256], spin[:, 0:128], spin[:])
                    k_chunk(0, 0)
                    q_chunk(0, 0)
                    if VHEAD:
                        v_chunk(0)
                        # kt chunks and V blocks feed into the first attention
                        # block just ahead of the strips that read them.
                        attn(0, 0, feeders={
                            0: [lambda: k_chunk(0, 1)],
                            2: [lambda: v_chunk(1)],
                            4: [lambda: k_chunk(0, 2)],
                            6: [lambda: v_chunk(2)],
                            8: [lambda: k_chunk(0, 3)],
                            10: [lambda: v_chunk(3)],
                        })
                    else:
                        for c in range(1, 4):
                            k_chunk(0, c)
                        for c in range(4):
                            v_chunk(c)
                        attn(0, 0)
                    q_chunk(0, 1)
                    attn(0, 1, feeders={0: [lambda: k_chunk(1, 0)],
                                        8: [lambda: k_chunk(1, 1)]})
                    q_chunk(0, 2)
                    attn(0, 2, feeders={0: [lambda: k_chunk(1, 2)],
                                        8: [lambda: k_chunk(1, 3)]})
                    q_chunk(0, 3)
                    attn(0, 3)
                    q_chunk(1, 0)
                    attn(1, 0)
                    q_chunk(1, 1)
                    proj(range(0, 4))
                    attn(1, 1)
                    q_chunk(1, 2)
                    proj(range(4, 8))
                    attn(1, 2)
                    q_chunk(1, 3)
                    proj(range(8, 12))
                    attn(1, 3)
                    proj(range(12, 16))
                if DEBUG:
                    for nm, sb in (("dq", qt_sb), ("dk", kt_sb),
                                   ("dv", vaug_sb), ("do", ot_sb),
                                   ("dd", dden_sb), ("du", dunn_sb),
                                   ("de", dexp_sb)):
                        nc.sync.dma_start(out=dumps[nm][:, :], in_=sb[:])
    nc.finalize()
    return nc


def make_in_maps(x, Wq, Wk, Wv, Wo):
    bf = np.float16
    f = np.float32
    x = np.asarray(x, f)
    Wq, Wk, Wv, Wo = (np.asarray(a, f) for a in (Wq, Wk, Wv, Wo))
    in_maps = []
    xTs = [np.ascontiguousarray(x[b].T).astype(bf) for b in range(B)]
    for c in range(N_CORES):
        b, hb = divmod(c, N_CORES // B)
        cols = slice(hb * DL, (hb + 1) * DL)
        in_maps.append({
            "xT": xTs[b],
            "Wq": np.ascontiguousarray(Wq[:, cols]).astype(bf),
            "Wk": np.ascontiguousarray(Wk[:, cols]).astype(bf),
            "Wv": np.ascontiguousarray(Wv[:, cols]).astype(bf),
            "Wo": (np.ascontiguousarray(Wo[cols, :]) * f(1.0 / 32.0)).astype(bf),
        })
    return in_maps


def run(inputs, trace=False):
    nc = build_nc()
    in_maps = make_in_maps(**inputs)
    res = run_bass_kernel_spmd(nc, in_maps, list(range(N_CORES)), trace=trace)
    yps = [res.results[c]["Yp"] for c in range(N_CORES)]
    out = np.empty((B, S, D), np.float32)
    cpb = N_CORES // B
    for b in range(B):
        out[b] = sum(np.asarray(yp, np.float32) for yp in yps[b * cpb:(b + 1) * cpb])
    return out, res


def kernel(**inputs):
    out, _ = run(inputs, trace=False)
    return out
